# revision 32
# baseline (speedup 1.0000x reference)
"""Paged GQA decode attention on 8 TRN2 NeuronCores.

Sharding: tensor-parallel over heads. Core m owns kv head m and query
heads [4m, 4m+4). block_tables / slot_mapping are applied on the host,
which gathers each sequence's valid cache prefix (new k/v token
scattered in) into dense per-core layouts; context_lens are baked into
the (shared SPMD) graph as static loop bounds. No collectives.

Per-core HBM layout (host-prepared from the full inputs):
  qt  [128, 64]          bf16, qt[d, 4b+h] = q[b, 4m+h, d] * scale
  kt  [128, TTOT*128]    fp8 e3m4, K^T, per-seq column ranges
  vt  [128, TTOT, 128]   fp8 e3m4, V in 128-token tiles, partition =
                         token-within-tile, free = head dim
Outputs (host finishes the softmax normalization + transpose):
  o   [128, 64]  f32     o[d, 4i+h] = unnormalized attn out, seq order i
  den [1, 4*TTOT] f32    per-tile softmax partial sums (ones-matmul)

Device, per sequence b with S = context_lens[b], nt = ceil(S/128):
  scoresT[s, 4t+h] via matmul(lhsT=K-tile fp8 [128d, T], rhs=qt_b bf16)
  exp on ScalarE (PSUM f32 -> SBUF bf16), garbage rows of the last
  partial tile pre-zeroed so the denominator matmul can contract all
  128 partitions
  den partials: matmul(lhsT=ones [128,1], rhs=pt [128, 4nt])
  oT[128d, 4h] += matmul(lhsT=V-tile fp8 [T, 128], rhs=pt-tile [T, 4])
Tensor work per tile is one 128-row LDWEIGHTS + a 4-column stream for
both matmuls; fp8 K/V halves HBM traffic vs bf16 (the bottleneck, with
the per-core HBM read bandwidth capping out near ~310 GB/s).

Schedule: K on the Sync HWDGE queue, V on the GpSimd SWDGE queue (the
Scalar engine stays free so descriptor-gen never delays the exp chain);
transfers grouped to ~5KB/partition so per-transfer semaphore-rotation
bubbles amortize; sequences ordered tiny-first / small-last; per-seq
work software-pipelined two sequences deep so the in-order PE stream
never stalls on a pending exp.
"""

import numpy as np

B = 16
H = 32
HKV = 8
D = 128
BLOCK = 256
MAX_KV = 4096
N_CORES = 8
HPC = H // N_CORES  # query heads per core
SCALE = np.float32(1.0 / np.sqrt(D))

from ml_dtypes import bfloat16 as _bf16
from ml_dtypes import float8_e3m4 as _f8

_graph_cache: dict = {}


def _plan(context_lens):
    """Order sequences for pipelined per-seq DMA. Returns (order, nts,
    offs, ttot): nts[b]=ceil(S/128), offs[b]=tile offset of b."""
    nts = [max(1, -(-int(s) // 128)) for s in context_lens]
    asc = sorted(range(B), key=lambda b: nts[b])
    # two tiny sequences first (instant pipeline fill); the two biggest
    # mid-stream so their long dependency chains overlap DMA; medium
    # sequences late; two small ones last so only short chains remain
    # after the final DMA byte
    # two tiny sequences first (instant pipeline fill), descending
    # middle so the big sequences' long chains overlap the DMA stream,
    # two small ones last so only short chains remain after the final
    # DMA byte
    order = tuple(asc[0:2] + asc[:3:-1] + asc[2:4])
    offs = {}
    off = 0
    for b in order:
        offs[b] = off
        off += nts[b]
    return order, tuple(nts), offs, off


def _build(context_lens):
    import concourse.bacc as bacc
    import concourse.mybir as mybir
    import concourse.tile as tile

    f32 = mybir.dt.float32
    bf16 = mybir.dt.bfloat16
    f8 = mybir.dt.float8e3
    order, nts, offs, ttot = _plan(context_lens)
    nc = bacc.Bacc(None, target_bir_lowering=False)

    qt_ext = nc.declare_dram_parameter("qt", [D, B * HPC], bf16, isOutput=False)
    kt_ext = nc.declare_dram_parameter("kt", [D, ttot * 128], f8, isOutput=False)
    vt_ext = nc.declare_dram_parameter("vt", [128, ttot, D], f8, isOutput=False)
    o_ext = nc.declare_dram_parameter("o", [D, B * HPC], f32, isOutput=True)
    den_ext = nc.declare_dram_parameter("den", [1, HPC * ttot], f32, isOutput=True)

    with tile.TileContext(nc) as tc:
        with (
            tc.tile_pool(name="const", bufs=1) as const_pool,
            tc.tile_pool(name="ps_s", bufs=4, space="PSUM") as ps_s_pool,
            tc.tile_pool(name="ps_o", bufs=2, space="PSUM") as ps_o_pool,
            tc.tile_pool(name="ps_d", bufs=2, space="PSUM") as ps_d_pool,
        ):
            qt = const_pool.tile([D, B * HPC], bf16)
            nc.scalar.dma_start(qt[:], qt_ext[:])
            ones = const_pool.tile([128, 1], bf16)
            nc.vector.memset(ones[:], 1.0)
            o_all = const_pool.tile([D, B * HPC], f32)
            den_all = const_pool.tile([1, HPC * ttot], f32)

            kt_all = const_pool.tile([D, ttot * 128], f8)
            vt_all = const_pool.tile([128, ttot, D], f8)
            kts = {b: kt_all[:, offs[b] * 128 : (offs[b] + nts[b]) * 128] for b in order}
            vts = {b: vt_all[:, offs[b] : offs[b] + nts[b], :] for b in order}
            pts = {}
            for b in order:
                nt = nts[b]
                pts[b] = const_pool.tile([128, HPC * nt], bf16, name=f"pt{b}")
                S = int(context_lens[b])
                T = S - (nt - 1) * 128
                if T < 128:
                    # zero the last partial tile's columns so the
                    # ones-matmul can contract all 128 partitions; the
                    # exp later overwrites rows [0:T] with valid values
                    # (partition ranges not starting at 0 are limited
                    # to 32 partitions, so clear all 128 rows)
                    nc.vector.memset(pts[b][:, HPC * nt - HPC : HPC * nt], 0.0)

            # Grouped K/V loads into the arenas: fewer transfers per
            # queue hides the per-transfer DGE semaphore-rotation bubble
            # (~0.7us each).  Region-level hazard tracking lets per-seq
            # matmuls start as soon as their group lands.  K on the Sync
            # HWDGE queue, V on the GpSimd SWDGE queue; Scalar stays
            # free for the exp chain.
            def grouped(cap):
                gs, cur, cb = [], [], 0
                for b in order:
                    sz = nts[b] * 128
                    if cur and cb + sz > cap:
                        gs.append(cur)
                        cur, cb = [], 0
                    cur.append(b)
                    cb += sz
                gs.append(cur)
                return gs

            def span(g):
                return offs[g[0]], offs[g[-1]] + nts[g[-1]]

            # K-first scheduling: each queue sends all of its K groups,
            # then its V groups.  All scores/exp chains complete while V
            # still streams, so after the last DMA byte only the final
            # (small) PV chain remains.  Whatever lands LAST defines the
            # tail, and a late V costs just one PV, not a whole
            # scores->exp->PV chain.  Queues balanced by measured rate
            # (Sync HWDGE ~145 GB/s, GpSimd SWDGE ~162 GB/s).
            qtime = {0: 0.0, 1: 0.0}  # 0 = gpsimd, 1 = sync
            rate = {0: 162.0, 1: 158.0}

            def assign(lo, hi):
                sz = (hi - lo) * 128 * 128 / 1000.0
                q = 0 if qtime[0] + sz / rate[0] <= qtime[1] + sz / rate[1] else 1
                qtime[q] += sz / rate[q]
                return (nc.gpsimd if q == 0 else nc.sync), qtime[q]

            for g in grouped(8000):
                lo, hi = span(g)
                eng, _ = assign(lo, hi)
                eng.dma_start(
                    kt_all[:, lo * 128 : hi * 128],
                    kt_ext[:, lo * 128 : hi * 128],
                )
            v_landing = []
            for g in grouped(2200):
                lo, hi = span(g)
                eng, t_fin = assign(lo, hi)
                eng.dma_start(vt_all[:, lo:hi, :], vt_ext[:, lo:hi, :])
                v_landing.append((t_fin, g))
            # sequences in predicted V-arrival order: phase B consumes V
            # exactly as it lands, so the in-order PE stream never
            # stalls on a V group that arrives later than a queue-mate
            arrival = [b for _, g in sorted(v_landing, key=lambda x: x[0]) for b in g]

            # phase A emits scores+exp per sequence in K-arrival order
            # (no PE instruction waits on the Scalar exp, so the stream
            # never bubbles); phase B emits den+PV+copies in V-arrival
            # order
            def scores(b):
                S = int(context_lens[b])
                nt = nts[b]
                ps_s = ps_s_pool.tile([128, 128], mybir.dt.float32, tag="s")
                for t in range(nt):
                    T = min(128, S - t * 128)
                    nc.tensor.matmul(
                        ps_s[0:T, HPC * t : HPC * t + HPC],
                        kts[b][:, t * 128 : t * 128 + T],
                        qt[:, HPC * b : HPC * b + HPC],
                        start=True,
                        stop=True,
                    )
                return ps_s

            def exp_emit(b, ps_s):
                S = int(context_lens[b])
                nt = nts[b]
                pt = pts[b]
                T = S - (nt - 1) * 128
                if T < 128:
                    if nt > 1:
                        nc.scalar.activation(
                            pt[:, 0 : HPC * (nt - 1)],
                            ps_s[:, 0 : HPC * (nt - 1)],
                            mybir.ActivationFunctionType.Exp,
                        )
                    nc.scalar.activation(
                        pt[0:T, HPC * (nt - 1) : HPC * nt],
                        ps_s[0:T, HPC * (nt - 1) : HPC * nt],
                        mybir.ActivationFunctionType.Exp,
                    )
                else:
                    nc.scalar.activation(
                        pt[:, 0 : HPC * nt],
                        ps_s[:, 0 : HPC * nt],
                        mybir.ActivationFunctionType.Exp,
                    )

            def finish(b, j):
                S = int(context_lens[b])
                nt = nts[b]
                off = offs[b]
                pt = pts[b]
                ps_d = ps_d_pool.tile([1, 128], mybir.dt.float32, tag="d")
                nc.tensor.matmul(
                    ps_d[0:1, 0 : HPC * nt],
                    ones[:, 0:1],
                    pt[:, 0 : HPC * nt],
                    start=True,
                    stop=True,
                )
                ps_o = ps_o_pool.tile([D, HPC], mybir.dt.float32, tag="o")
                for t in range(nt):
                    T = min(128, S - t * 128)
                    nc.tensor.matmul(
                        ps_o[:, :],
                        vts[b][0:T, t, :],
                        pt[0:T, HPC * t : HPC * t + HPC],
                        start=(t == 0),
                        stop=(t == nt - 1),
                    )
                nc.vector.tensor_copy(o_all[:, HPC * j : HPC * j + HPC], ps_o[:, :])
                # den copy on Scalar (idle once the exps are done): the
                # ps_o and ps_d recycle chains then wait on different
                # engines, halving the per-finish copy serialization
                nc.scalar.copy(
                    den_all[0:1, HPC * off : HPC * off + HPC * nt],
                    ps_d[0:1, 0 : HPC * nt],
                )

            for b in order:
                exp_emit(b, scores(b))
            for j, b in enumerate(arrival):
                finish(b, j)
                if j == 9:
                    # first output wave: overlap the writeback of the
                    # first ten finished sequences with remaining PV
                    nc.sync.dma_start(
                        o_ext[:, 0 : 10 * HPC], o_all[:, 0 : 10 * HPC]
                    )

            nc.sync.dma_start(
                o_ext[:, 10 * HPC : B * HPC], o_all[:, 10 * HPC : B * HPC]
            )
            # den in one final wave on Scalar (its exps are long done)
            # so the two last descriptor-gens run in parallel
            nc.scalar.dma_start(den_ext[:], den_all[:])

        arrival_order = arrival

    nc.compile()
    return nc, arrival_order, nts, offs, ttot


def _prep_inputs(inputs, order, nts, offs, ttot):
    q = np.asarray(inputs["q"], dtype=np.float32)
    k = np.asarray(inputs["k"], dtype=np.float32)
    v = np.asarray(inputs["v"], dtype=np.float32)
    k_cache = np.asarray(inputs["k_cache"], dtype=np.float32)
    v_cache = np.asarray(inputs["v_cache"], dtype=np.float32)
    context_lens = np.asarray(inputs["context_lens"])
    block_tables = np.asarray(inputs["block_tables"])
    slot_mapping = np.asarray(inputs["slot_mapping"])
    nslot = k_cache.shape[0] * k_cache.shape[1]

    # per-seq gathered slot indices (ceil128 of context), block_tables applied
    slot_idx = {}
    for b in range(B):
        ncols = nts[b] * 128
        nblk = -(-ncols // BLOCK)
        blocks = block_tables[b, :nblk].astype(np.int64)
        idx = (blocks[:, None] * BLOCK + np.arange(BLOCK)[None, :]).reshape(-1)[:ncols]
        slot_idx[b] = idx

    in_maps = []
    for m in range(N_CORES):
        kc = k_cache[:, :, m, :].reshape(nslot, D)  # strided view
        vc = v_cache[:, :, m, :].reshape(nslot, D)
        kt = np.empty((D, ttot * 128), dtype=_f8)
        vt = np.empty((128, ttot, D), dtype=_f8)
        for b in range(B):
            idx = slot_idx[b]
            kg = kc[idx]  # [ncols, 128] gather (copy)
            vg = vc[idx]
            # scatter the new token (reference's _store_kvcache)
            sm = int(slot_mapping[b])
            if sm >= 0:
                pos = np.nonzero(idx == sm)[0]
                if pos.size:
                    kg[pos[0]] = k[b, m]
                    vg[pos[0]] = v[b, m]
            off = offs[b]
            nt = nts[b]
            kt[:, off * 128 : off * 128 + nt * 128] = kg.T.astype(_f8)
            vt[:, off : off + nt, :] = (
                vg.reshape(nt, 128, D).transpose(1, 0, 2).astype(_f8)
            )
        qt = np.ascontiguousarray(
            (q[:, HPC * m : HPC * m + HPC, :].reshape(B * HPC, D) * SCALE).T
        ).astype(_bf16)
        in_maps.append({"qt": qt, "kt": kt, "vt": vt})
    return in_maps


def _run(inputs: dict, trace: bool = False, tmpdir: str | None = None):
    from concourse.bass_utils import run_bass_kernel_spmd

    context_lens = np.asarray(inputs["context_lens"])
    key = tuple(int(x) for x in context_lens)
    cached = _graph_cache.get(key)
    if cached is None:
        cached = _build(context_lens)
        _graph_cache[key] = cached
    nc, order, nts, offs, ttot = cached

    in_maps = _prep_inputs(inputs, order, nts, offs, ttot)
    res = run_bass_kernel_spmd(
        nc, in_maps, list(range(N_CORES)), trace=trace, tmpdir=tmpdir
    )

    out = np.empty((B, 1, H, D), dtype=np.float32)
    for m in range(N_CORES):
        om = np.asarray(res.results[m]["o"])  # [128, 64] f32, o^T
        dm = np.asarray(res.results[m]["den"]).reshape(-1)  # [4*ttot]
        for i, b in enumerate(order):
            off = offs[b]
            nt = nts[b]
            den = dm[HPC * off : HPC * off + HPC * nt].reshape(nt, HPC).sum(axis=0)
            oT = om[:, HPC * i : HPC * i + HPC]  # [128, 4]
            out[b, 0, HPC * m : HPC * m + HPC, :] = (oT / den[None, :]).T
    return out, res


def kernel(**inputs) -> np.ndarray:
    out, _ = _run(inputs, trace=False)
    return out


# revision 34
# speedup vs baseline: 1.0152x; 1.0152x over previous
"""Paged GQA decode attention on 8 TRN2 NeuronCores.

Sharding: tensor-parallel over heads. Core m owns kv head m and query
heads [4m, 4m+4). block_tables / slot_mapping are applied on the host,
which gathers each sequence's valid cache prefix (new k/v token
scattered in) into dense per-core layouts; context_lens are baked into
the (shared SPMD) graph as static loop bounds. No collectives.

Per-core HBM layout (host-prepared from the full inputs):
  qt  [128, 64]          bf16, qt[d, 4b+h] = q[b, 4m+h, d] * scale
  kt  [128, TTOT*128]    fp8 e3m4, K^T, per-seq column ranges
  vt  [128, TTOT, 128]   fp8 e3m4, V in 128-token tiles, partition =
                         token-within-tile, free = head dim
Outputs (host finishes the softmax normalization + transpose):
  o   [128, 64]  f32     o[d, 4i+h] = unnormalized attn out, seq order i
  den [1, 4*TTOT] f32    per-tile softmax partial sums (ones-matmul)

Device, per sequence b with S = context_lens[b], nt = ceil(S/128):
  scoresT[s, 4t+h] via matmul(lhsT=K-tile fp8 [128d, T], rhs=qt_b bf16)
  exp on ScalarE (PSUM f32 -> SBUF bf16), garbage rows of the last
  partial tile pre-zeroed so the denominator matmul can contract all
  128 partitions
  den partials: matmul(lhsT=ones [128,1], rhs=pt [128, 4nt])
  oT[128d, 4h] += matmul(lhsT=V-tile fp8 [T, 128], rhs=pt-tile [T, 4])
Tensor work per tile is one 128-row LDWEIGHTS + a 4-column stream for
both matmuls; fp8 K/V halves HBM traffic vs bf16 (the bottleneck, with
the per-core HBM read bandwidth capping out near ~310 GB/s).

Schedule: K on the Sync HWDGE queue, V on the GpSimd SWDGE queue (the
Scalar engine stays free so descriptor-gen never delays the exp chain);
transfers grouped to ~5KB/partition so per-transfer semaphore-rotation
bubbles amortize; sequences ordered tiny-first / small-last; per-seq
work software-pipelined two sequences deep so the in-order PE stream
never stalls on a pending exp.
"""

import numpy as np

B = 16
H = 32
HKV = 8
D = 128
BLOCK = 256
MAX_KV = 4096
N_CORES = 8
HPC = H // N_CORES  # query heads per core
SCALE = np.float32(1.0 / np.sqrt(D))

from ml_dtypes import bfloat16 as _bf16
from ml_dtypes import float8_e3m4 as _f8

_graph_cache: dict = {}


def _plan(context_lens):
    """Order sequences for pipelined per-seq DMA. Returns (order, nts,
    offs, ttot): nts[b]=ceil(S/128), offs[b]=tile offset of b."""
    nts = [max(1, -(-int(s) // 128)) for s in context_lens]
    asc = sorted(range(B), key=lambda b: nts[b])
    # two tiny sequences first (instant pipeline fill); the two biggest
    # mid-stream so their long dependency chains overlap DMA; medium
    # sequences late; two small ones last so only short chains remain
    # after the final DMA byte
    # two tiny sequences first (instant pipeline fill), descending
    # middle so the big sequences' long chains overlap the DMA stream,
    # two small ones last so only short chains remain after the final
    # DMA byte
    order = tuple(asc[0:2] + asc[:3:-1] + asc[2:4])
    offs = {}
    off = 0
    for b in order:
        offs[b] = off
        off += nts[b]
    return order, tuple(nts), offs, off


def _build(context_lens):
    import concourse.bacc as bacc
    import concourse.mybir as mybir
    import concourse.tile as tile

    f32 = mybir.dt.float32
    bf16 = mybir.dt.bfloat16
    f8 = mybir.dt.float8e3
    order, nts, offs, ttot = _plan(context_lens)
    nc = bacc.Bacc(None, target_bir_lowering=False)

    qt_ext = nc.declare_dram_parameter("qt", [D, B * HPC], bf16, isOutput=False)
    kt_ext = nc.declare_dram_parameter("kt", [D, ttot * 128], f8, isOutput=False)
    vt_ext = nc.declare_dram_parameter("vt", [128, ttot, D], f8, isOutput=False)
    o_ext = nc.declare_dram_parameter("o", [D, B * HPC], f32, isOutput=True)
    den_ext = nc.declare_dram_parameter("den", [1, HPC * ttot], f32, isOutput=True)

    with tile.TileContext(nc) as tc:
        with (
            tc.tile_pool(name="const", bufs=1) as const_pool,
            tc.tile_pool(name="ps_s", bufs=4, space="PSUM") as ps_s_pool,
            tc.tile_pool(name="ps_o", bufs=2, space="PSUM") as ps_o_pool,
            tc.tile_pool(name="ps_d", bufs=2, space="PSUM") as ps_d_pool,
        ):
            qt = const_pool.tile([D, B * HPC], bf16)
            nc.scalar.dma_start(qt[:], qt_ext[:])
            ones = const_pool.tile([128, 1], bf16)
            nc.vector.memset(ones[:], 1.0)
            o_all = const_pool.tile([D, B * HPC], f32)
            den_all = const_pool.tile([1, HPC * ttot], f32)

            kt_all = const_pool.tile([D, ttot * 128], f8)
            vt_all = const_pool.tile([128, ttot, D], f8)
            kts = {b: kt_all[:, offs[b] * 128 : (offs[b] + nts[b]) * 128] for b in order}
            vts = {b: vt_all[:, offs[b] : offs[b] + nts[b], :] for b in order}
            pts = {}
            for b in order:
                nt = nts[b]
                pts[b] = const_pool.tile([128, HPC * nt], bf16, name=f"pt{b}")
                S = int(context_lens[b])
                T = S - (nt - 1) * 128
                if T < 128:
                    # zero the last partial tile's columns so the
                    # ones-matmul can contract all 128 partitions; the
                    # exp later overwrites rows [0:T] with valid values
                    # (partition ranges not starting at 0 are limited
                    # to 32 partitions, so clear all 128 rows)
                    nc.vector.memset(pts[b][:, HPC * nt - HPC : HPC * nt], 0.0)

            # Grouped K/V loads into the arenas: fewer transfers per
            # queue hides the per-transfer DGE semaphore-rotation bubble
            # (~0.7us each).  Region-level hazard tracking lets per-seq
            # matmuls start as soon as their group lands.  K on the Sync
            # HWDGE queue, V on the GpSimd SWDGE queue; Scalar stays
            # free for the exp chain.
            def grouped(cap):
                gs, cur, cb = [], [], 0
                for b in order:
                    sz = nts[b] * 128
                    if cur and cb + sz > cap:
                        gs.append(cur)
                        cur, cb = [], 0
                    cur.append(b)
                    cb += sz
                gs.append(cur)
                return gs

            def span(g):
                return offs[g[0]], offs[g[-1]] + nts[g[-1]]

            # K-first scheduling: each queue sends all of its K groups,
            # then its V groups.  All scores/exp chains complete while V
            # still streams, so after the last DMA byte only the final
            # (small) PV chain remains.  Whatever lands LAST defines the
            # tail, and a late V costs just one PV, not a whole
            # scores->exp->PV chain.  Queues balanced by measured rate
            # (Sync HWDGE ~145 GB/s, GpSimd SWDGE ~162 GB/s).
            qtime = {0: 0.0, 1: 0.0}  # 0 = gpsimd, 1 = sync
            rate = {0: 162.0, 1: 158.0}

            def assign(lo, hi):
                sz = (hi - lo) * 128 * 128 / 1000.0
                q = 0 if qtime[0] + sz / rate[0] <= qtime[1] + sz / rate[1] else 1
                qtime[q] += sz / rate[q]
                return (nc.gpsimd if q == 0 else nc.sync), qtime[q]

            for g in grouped(8000):
                lo, hi = span(g)
                eng, _ = assign(lo, hi)
                eng.dma_start(
                    kt_all[:, lo * 128 : hi * 128],
                    kt_ext[:, lo * 128 : hi * 128],
                )
            v_landing = []
            v_groups = grouped(3000)
            # split the last two V groups per-seq: the final landings
            # gate the kernel tail, so spread them as finely as possible
            # (the extra descriptor-gen cost is absorbed mid-stream)
            v_groups = v_groups[:-2] + [[b] for g in v_groups[-2:] for b in g]
            for g in v_groups:
                lo, hi = span(g)
                eng, t_fin = assign(lo, hi)
                eng.dma_start(vt_all[:, lo:hi, :], vt_ext[:, lo:hi, :])
                v_landing.append((t_fin, g))
            # sequences in predicted V-arrival order: phase B consumes V
            # exactly as it lands, so the in-order PE stream never
            # stalls on a V group that arrives later than a queue-mate
            arrival = [b for _, g in sorted(v_landing, key=lambda x: x[0]) for b in g]

            # phase A emits scores+exp per sequence in K-arrival order
            # (no PE instruction waits on the Scalar exp, so the stream
            # never bubbles); phase B emits den+PV+copies in V-arrival
            # order
            def scores(b):
                S = int(context_lens[b])
                nt = nts[b]
                ps_s = ps_s_pool.tile([128, 128], mybir.dt.float32, tag="s")
                for t in range(nt):
                    T = min(128, S - t * 128)
                    nc.tensor.matmul(
                        ps_s[0:T, HPC * t : HPC * t + HPC],
                        kts[b][:, t * 128 : t * 128 + T],
                        qt[:, HPC * b : HPC * b + HPC],
                        start=True,
                        stop=True,
                    )
                return ps_s

            def exp_emit(b, ps_s):
                S = int(context_lens[b])
                nt = nts[b]
                pt = pts[b]
                T = S - (nt - 1) * 128
                if T < 128:
                    if nt > 1:
                        nc.scalar.activation(
                            pt[:, 0 : HPC * (nt - 1)],
                            ps_s[:, 0 : HPC * (nt - 1)],
                            mybir.ActivationFunctionType.Exp,
                        )
                    nc.scalar.activation(
                        pt[0:T, HPC * (nt - 1) : HPC * nt],
                        ps_s[0:T, HPC * (nt - 1) : HPC * nt],
                        mybir.ActivationFunctionType.Exp,
                    )
                else:
                    nc.scalar.activation(
                        pt[:, 0 : HPC * nt],
                        ps_s[:, 0 : HPC * nt],
                        mybir.ActivationFunctionType.Exp,
                    )

            def finish(b, j):
                S = int(context_lens[b])
                nt = nts[b]
                off = offs[b]
                pt = pts[b]
                ps_d = ps_d_pool.tile([1, 128], mybir.dt.float32, tag="d")
                nc.tensor.matmul(
                    ps_d[0:1, 0 : HPC * nt],
                    ones[:, 0:1],
                    pt[:, 0 : HPC * nt],
                    start=True,
                    stop=True,
                )
                ps_o = ps_o_pool.tile([D, HPC], mybir.dt.float32, tag="o")
                for t in range(nt):
                    T = min(128, S - t * 128)
                    nc.tensor.matmul(
                        ps_o[:, :],
                        vts[b][0:T, t, :],
                        pt[0:T, HPC * t : HPC * t + HPC],
                        start=(t == 0),
                        stop=(t == nt - 1),
                    )
                nc.vector.tensor_copy(o_all[:, HPC * j : HPC * j + HPC], ps_o[:, :])
                # den copy on Scalar (idle once the exps are done): the
                # ps_o and ps_d recycle chains then wait on different
                # engines, halving the per-finish copy serialization
                nc.scalar.copy(
                    den_all[0:1, HPC * off : HPC * off + HPC * nt],
                    ps_d[0:1, 0 : HPC * nt],
                )

            for b in order:
                exp_emit(b, scores(b))
            for j, b in enumerate(arrival):
                finish(b, j)
                if j == 9:
                    # first output wave: overlap the writeback of the
                    # first ten finished sequences with remaining PV
                    nc.sync.dma_start(
                        o_ext[:, 0 : 10 * HPC], o_all[:, 0 : 10 * HPC]
                    )

            nc.sync.dma_start(
                o_ext[:, 10 * HPC : B * HPC], o_all[:, 10 * HPC : B * HPC]
            )
            # den in one final wave on Scalar (its exps are long done)
            # so the two last descriptor-gens run in parallel
            nc.scalar.dma_start(den_ext[:], den_all[:])

        arrival_order = arrival

    nc.compile()
    return nc, arrival_order, nts, offs, ttot


def _prep_inputs(inputs, order, nts, offs, ttot):
    q = np.asarray(inputs["q"], dtype=np.float32)
    k = np.asarray(inputs["k"], dtype=np.float32)
    v = np.asarray(inputs["v"], dtype=np.float32)
    k_cache = np.asarray(inputs["k_cache"], dtype=np.float32)
    v_cache = np.asarray(inputs["v_cache"], dtype=np.float32)
    context_lens = np.asarray(inputs["context_lens"])
    block_tables = np.asarray(inputs["block_tables"])
    slot_mapping = np.asarray(inputs["slot_mapping"])
    nslot = k_cache.shape[0] * k_cache.shape[1]

    # per-seq gathered slot indices (ceil128 of context), block_tables applied
    slot_idx = {}
    for b in range(B):
        ncols = nts[b] * 128
        nblk = -(-ncols // BLOCK)
        blocks = block_tables[b, :nblk].astype(np.int64)
        idx = (blocks[:, None] * BLOCK + np.arange(BLOCK)[None, :]).reshape(-1)[:ncols]
        slot_idx[b] = idx

    in_maps = []
    for m in range(N_CORES):
        kc = k_cache[:, :, m, :].reshape(nslot, D)  # strided view
        vc = v_cache[:, :, m, :].reshape(nslot, D)
        kt = np.empty((D, ttot * 128), dtype=_f8)
        vt = np.empty((128, ttot, D), dtype=_f8)
        for b in range(B):
            idx = slot_idx[b]
            kg = kc[idx]  # [ncols, 128] gather (copy)
            vg = vc[idx]
            # scatter the new token (reference's _store_kvcache)
            sm = int(slot_mapping[b])
            if sm >= 0:
                pos = np.nonzero(idx == sm)[0]
                if pos.size:
                    kg[pos[0]] = k[b, m]
                    vg[pos[0]] = v[b, m]
            off = offs[b]
            nt = nts[b]
            kt[:, off * 128 : off * 128 + nt * 128] = kg.T.astype(_f8)
            vt[:, off : off + nt, :] = (
                vg.reshape(nt, 128, D).transpose(1, 0, 2).astype(_f8)
            )
        qt = np.ascontiguousarray(
            (q[:, HPC * m : HPC * m + HPC, :].reshape(B * HPC, D) * SCALE).T
        ).astype(_bf16)
        in_maps.append({"qt": qt, "kt": kt, "vt": vt})
    return in_maps


def _run(inputs: dict, trace: bool = False, tmpdir: str | None = None):
    from concourse.bass_utils import run_bass_kernel_spmd

    context_lens = np.asarray(inputs["context_lens"])
    key = tuple(int(x) for x in context_lens)
    cached = _graph_cache.get(key)
    if cached is None:
        cached = _build(context_lens)
        _graph_cache[key] = cached
    nc, order, nts, offs, ttot = cached

    in_maps = _prep_inputs(inputs, order, nts, offs, ttot)
    res = run_bass_kernel_spmd(
        nc, in_maps, list(range(N_CORES)), trace=trace, tmpdir=tmpdir
    )

    out = np.empty((B, 1, H, D), dtype=np.float32)
    for m in range(N_CORES):
        om = np.asarray(res.results[m]["o"])  # [128, 64] f32, o^T
        dm = np.asarray(res.results[m]["den"]).reshape(-1)  # [4*ttot]
        for i, b in enumerate(order):
            off = offs[b]
            nt = nts[b]
            den = dm[HPC * off : HPC * off + HPC * nt].reshape(nt, HPC).sum(axis=0)
            oT = om[:, HPC * i : HPC * i + HPC]  # [128, 4]
            out[b, 0, HPC * m : HPC * m + HPC, :] = (oT / den[None, :]).T
    return out, res


def kernel(**inputs) -> np.ndarray:
    out, _ = _run(inputs, trace=False)
    return out
